# revision 6
# baseline (speedup 1.0000x reference)
"""AgentGNN (2x CGConv + BN + residual + ReLU) on 8 TRN2 NeuronCores.

Self-contained: takes FULL inputs, shards 8 samples/core (data parallel),
runs a Bass/Tile kernel via run_bass_kernel_spmd, gathers FULL output.

Math: edges are fully-connected per 64-node sample and e_ij = c_i - c_j,
so  z_ij @ W.T + b  separates into per-node terms:
    alpha_i = x_i@Wa.T + c_i@Wc.T + b     (target part,  Wa = W[:, :F])
    beta_j  = x_j@Wb.T - c_j@Wc.T         (source part,  Wb = W[:, F:2F])
    msg_ij  = sigmoid(alpha_i + beta_j) * softplus(gamma_i + delta_j)
    agg_i   = sum_j msg_ij - msg_ii       (dense 64x64 incl. diag, minus diag)
BN (over all 4096 nodes, per feature) needs one cross-core AllReduce of
[sum, sumsq] per layer.

Engine mapping per layer (per core: 8 samples, pairwise = 8x[128,4096]):
  PE:  8 small projection matmuls (centers+bias folded in as K=3 term)
  DVE: outer-adds (broadcast-AP tensor_tensor), S*T multiply, j-reduce
  ACT: sigmoid; softplus as ln(1+exp(x)) (no softplus in this runtime's
       tables); table-set thrash avoided by dep-chaining ACT order
  GPSIMD: nothing (it shares its SBUF port with DVE - offload is zero-sum)
"""

import numpy as np

N_SAMPLES = 64
N_AGENTS = 64
N = N_SAMPLES * N_AGENTS          # 4096
F = 128
EDIM = 2
BN_EPS = 1e-5
N_CORES = 8
S_PC = N_SAMPLES // N_CORES       # 8 samples per core
NODES_PC = S_PC * N_AGENTS        # 512 nodes per core
GROUP = 4                         # samples per ACT table-set group

_CACHE = {}


def _build_nc():
    from concourse import bacc, mybir
    from concourse.tile import TileContext
    from concourse.tile_rust import add_dep_helper
    from concourse.bass_types import AP

    f32 = mybir.dt.float32
    AF = mybir.ActivationFunctionType
    OP = mybir.AluOpType
    AX = mybir.AxisListType

    nc = bacc.Bacc(trn_type="TRN2", target_bir_lowering=False, debug=False,
                   num_devices=N_CORES)

    xT = nc.declare_dram_parameter("xT", [F, NODES_PC], f32, isOutput=False)
    c3 = nc.declare_dram_parameter("c3", [EDIM + 1, NODES_PC], f32, isOutput=False)
    wnames = ["WaT", "WbT", "VaT", "VbT"]            # [128,128] x-side lhsT
    cnames = ["Wc3a", "Wc3b", "Vc3g", "Vc3d"]        # [3,128] centers+bias lhsT
    params = {}
    for l in (1, 2):
        for n in wnames:
            params[f"{n}{l}"] = nc.declare_dram_parameter(f"{n}{l}", [F, F], f32, isOutput=False)
        for n in cnames:
            params[f"{n}{l}"] = nc.declare_dram_parameter(f"{n}{l}", [EDIM + 1, F], f32, isOutput=False)
        params[f"g{l}"] = nc.declare_dram_parameter(f"g{l}", [F, 1], f32, isOutput=False)
        params[f"be{l}"] = nc.declare_dram_parameter(f"be{l}", [F, 1], f32, isOutput=False)
    yT = nc.declare_dram_parameter("yT", [F, NODES_PC], f32, isOutput=True)

    cc_in = {l: nc.dram_tensor(f"cc_in{l}", [F, 2], f32) for l in (1, 2)}
    cc_out = {l: nc.dram_tensor(f"cc_out{l}", [N_CORES * F, 2], f32, addr_space="Shared")
              for l in (1, 2)}

    with TileContext(nc) as tc:
        from contextlib import ExitStack
        with ExitStack() as ctx:
            io = ctx.enter_context(tc.tile_pool(name="io", bufs=1))
            wp = ctx.enter_context(tc.tile_pool(name="wp", bufs=1))
            node = ctx.enter_context(tc.tile_pool(name="node", bufs=1))
            pair = ctx.enter_context(tc.tile_pool(name="pair", bufs=1))
            psum = ctx.enter_context(tc.tile_pool(name="psum", bufs=1, space="PSUM"))
            small = ctx.enter_context(tc.tile_pool(name="small", bufs=1))

            # ---- load inputs & weights ----
            xt = io.tile([F, NODES_PC], f32, tag="xt")
            nc.sync.dma_start(xt[:], xT.ap()[:, :])
            c3t = io.tile([EDIM + 1, NODES_PC], f32, tag="c3t")
            nc.sync.dma_start(c3t[:], c3.ap()[:, :])
            wt = {}
            for l in (1, 2):
                for n in wnames:
                    t = wp.tile([F, F], f32, tag=f"{n}{l}")
                    nc.sync.dma_start(t[:], params[f"{n}{l}"].ap()[:, :])
                    wt[f"{n}{l}"] = t
                for n in cnames:
                    t = wp.tile([EDIM + 1, F], f32, tag=f"{n}{l}")
                    nc.sync.dma_start(t[:], params[f"{n}{l}"].ap()[:, :])
                    wt[f"{n}{l}"] = t
                for n in ("g", "be"):
                    t = wp.tile([F, 1], f32, tag=f"{n}{l}")
                    nc.sync.dma_start(t[:], params[f"{n}{l}"].ap()[:, :])
                    wt[f"{n}{l}"] = t

            def layer(l, x_in, x_out):
                # ---- node projections: alpha/beta/gamma/delta [128, 512] ----
                projs = []
                for pi, (wx, wc) in enumerate(zip(wnames, cnames)):
                    ps = psum.tile([F, NODES_PC], f32, tag=f"ps{pi}")
                    nc.tensor.matmul(ps[:], wt[f"{wx}{l}"][:], x_in[:],
                                     start=True, stop=False)
                    nc.tensor.matmul(ps[:], wt[f"{wc}{l}"][:], c3t[:],
                                     start=False, stop=True)
                    sb = node.tile([F, NODES_PC], f32, tag=f"proj{pi}")
                    nc.scalar.copy(sb[:], ps[:])
                    projs.append(sb)
                al, be_, ga, de = projs

                agg = node.tile([F, NODES_PC], f32, tag="agg")

                # ---- pairwise, one sample per chunk ----
                act_chain = []   # enforced ACT execution order

                def act(*args, **kw):
                    i = nc.scalar.activation(*args, **kw)
                    if act_chain:
                        add_dep_helper(i.ins, act_chain[-1].ins,
                                       reason="act set grouping")
                    act_chain.append(i)
                    return i

                for g in range(S_PC // GROUP):
                    cs = range(g * GROUP, (g + 1) * GROUP)
                    p1s, p2s = {}, {}
                    for c in cs:
                        sl = slice(c * N_AGENTS, (c + 1) * N_AGENTS)
                        a_bc = al[:, sl].broadcast_to([F, N_AGENTS, N_AGENTS])
                        b_bc = be_[:, sl].rearrange("p (o j) -> p o j", o=1) \
                            .broadcast_to([F, N_AGENTS, N_AGENTS])
                        g_bc = ga[:, sl].broadcast_to([F, N_AGENTS, N_AGENTS])
                        d_bc = de[:, sl].rearrange("p (o j) -> p o j", o=1) \
                            .broadcast_to([F, N_AGENTS, N_AGENTS])
                        p1 = pair.tile([F, N_AGENTS, N_AGENTS], f32, tag=f"p1_{c % GROUP}")
                        nc.vector.tensor_tensor(p1[:], a_bc, b_bc, op=OP.add)
                        p2 = pair.tile([F, N_AGENTS, N_AGENTS], f32, tag=f"p2_{c % GROUP}")
                        nc.vector.tensor_tensor(p2[:], g_bc, d_bc, op=OP.add)
                        p1s[c], p2s[c] = p1, p2
                    for c in cs:
                        act(p1s[c][:], p1s[c][:], AF.Sigmoid)
                    for c in cs:
                        act(p2s[c][:], p2s[c][:], AF.Exp)
                    for c in cs:
                        act(p2s[c][:], p2s[c][:], AF.Ln, bias=1.0)
                    for c in cs:
                        # M = S*T in place over p1
                        nc.vector.tensor_tensor(p1s[c][:], p1s[c][:], p2s[c][:],
                                                op=OP.mult)
                    for c in cs:
                        sl = slice(c * N_AGENTS, (c + 1) * N_AGENTS)
                        nc.vector.tensor_reduce(agg[:, sl], p1s[c][:],
                                                axis=AX.X, op=OP.add)
                        # subtract the self-edge messages: diagonal of M
                        m = p1s[c]
                        diag = AP(tensor=m[:].tensor, offset=m[:].offset,
                                  ap=[[4096, F], [65, N_AGENTS]])
                        nc.vector.tensor_tensor(agg[:, sl], agg[:, sl], diag,
                                                op=OP.subtract)

                # ---- BN stats: per-feature sum & sumsq over this core ----
                ssum = small.tile([F, 1], f32, tag="ssum")
                nc.vector.tensor_reduce(ssum[:], agg[:], axis=AX.X, op=OP.add)
                trash = node.tile([F, NODES_PC], f32, tag="trash")
                ssq = small.tile([F, 1], f32, tag="ssq")
                act(trash[:], agg[:], AF.Square, accum_out=ssq[:])

                dsum = nc.sync.dma_start(cc_in[l].ap()[:, 0:1], ssum[:])
                dsq = nc.sync.dma_start(cc_in[l].ap()[:, 1:2], ssq[:])
                ar = nc.gpsimd.collective_compute(
                    "AllGather", mybir.AluOpType.bypass,
                    replica_groups=[list(range(N_CORES))],
                    ins=[cc_in[l].ap().opt()], outs=[cc_out[l].ap().opt()])
                add_dep_helper(ar.ins, dsum.ins, reason="cc reads cc_in")
                add_dep_helper(ar.ins, dsq.ins, reason="cc reads cc_in")
                # gathered stats: [8*128, 2] -> view [128, 2, 8] and reduce
                gath = small.tile([F, 2, N_CORES], f32, tag="gath")
                din = nc.sync.dma_start(
                    gath[:], cc_out[l].ap().rearrange("(r p) c -> p c r", r=N_CORES))
                add_dep_helper(din.ins, ar.ins, reason="dma reads cc_out")
                gst = small.tile([F, 2], f32, tag="gst")
                nc.vector.tensor_reduce(gst[:], gath[:], axis=AX.X, op=OP.add)

                # ---- BN apply + residual + relu ----
                mean = small.tile([F, 1], f32, tag="mean")
                nc.vector.tensor_scalar(mean[:], gst[:, 0:1], 1.0 / N, None, op0=OP.mult)
                ex2 = small.tile([F, 1], f32, tag="ex2")
                nc.vector.tensor_scalar(ex2[:], gst[:, 1:2], 1.0 / N, None, op0=OP.mult)
                var = small.tile([F, 1], f32, tag="var")
                nc.vector.tensor_tensor(var[:], mean[:], mean[:], op=OP.mult)
                nc.vector.tensor_tensor(var[:], ex2[:], var[:], op=OP.subtract)
                nc.vector.tensor_scalar(var[:], var[:], BN_EPS, None, op0=OP.add)
                lnv = small.tile([F, 1], f32, tag="lnv")
                act(lnv[:], var[:], AF.Ln, bias=0.0)
                rstd = small.tile([F, 1], f32, tag="rstd")
                act(rstd[:], lnv[:], AF.Exp, bias=0.0, scale=-0.5)
                scal = small.tile([F, 1], f32, tag="scal")
                nc.vector.tensor_tensor(scal[:], rstd[:], wt[f"g{l}"][:], op=OP.mult)
                shift = small.tile([F, 1], f32, tag="shift")
                nc.vector.tensor_tensor(shift[:], mean[:], scal[:], op=OP.mult)
                nc.vector.tensor_tensor(shift[:], wt[f"be{l}"][:], shift[:], op=OP.subtract)
                nc.vector.tensor_scalar(x_out[:], agg[:], scal[:, 0:1], shift[:, 0:1],
                                        op0=OP.mult, op1=OP.add)
                nc.vector.tensor_tensor(x_out[:], x_out[:], x_in[:], op=OP.add)
                act(x_out[:], x_out[:], AF.Relu)

            x1 = io.tile([F, NODES_PC], f32, tag="x1")
            layer(1, xt, x1)
            x2 = io.tile([F, NODES_PC], f32, tag="x2")
            layer(2, x1, x2)
            nc.sync.dma_start(yT.ap()[:, :], x2[:])

    nc.compile()
    return nc


def _get_nc():
    if "nc" not in _CACHE:
        _CACHE["nc"] = _build_nc()
    return _CACHE["nc"]


def _canonical_edge_ok(src, dst):
    idx = np.arange(N_AGENTS)
    rows = np.repeat(idx, N_AGENTS)
    cols = np.tile(idx, N_AGENTS)
    m = rows != cols
    rows, cols = rows[m], cols[m]
    offs = (np.arange(N_SAMPLES) * N_AGENTS)[:, None]
    csrc = (rows[None, :] + offs).ravel().astype(np.int64)
    cdst = (cols[None, :] + offs).ravel().astype(np.int64)
    if src.shape != csrc.shape:
        return False
    key = np.sort(src.astype(np.int64) * N + dst.astype(np.int64))
    ckey = np.sort(csrc * N + cdst)
    return bool(np.array_equal(key, ckey))


def _numpy_fallback(gnn_in, centers, src, dst, Ws_all):
    def sig(x):
        return 1.0 / (1.0 + np.exp(-x))

    def sp(x):
        return np.log1p(np.exp(-np.abs(x))) + np.maximum(x, 0.0)

    x = gnn_in.astype(np.float64)
    e = (centers[dst] - centers[src]).astype(np.float64)
    for (Wf, bf, Wsm, bs, g, be) in Ws_all:
        z = np.concatenate([x[dst], x[src], e], axis=-1)
        msg = sig(z @ Wf.T + bf) * sp(z @ Wsm.T + bs)
        agg = np.zeros_like(x)
        np.add.at(agg, dst, msg)
        mean = agg.mean(0)
        var = agg.var(0)
        agg = (agg - mean) / np.sqrt(var + BN_EPS) * g + be
        x = np.maximum(agg + x, 0.0)
    return x.astype(np.float32)


def _host_weights(Wf, bf, Ws, bs):
    """lhsT forms for the projection matmuls."""
    WaT = np.ascontiguousarray(Wf[:, :F].T)
    WbT = np.ascontiguousarray(Wf[:, F:2 * F].T)
    Wc = Wf[:, 2 * F:2 * F + EDIM].T           # [2, 128]
    Wc3a = np.ascontiguousarray(np.concatenate([Wc, bf[None, :]], 0))
    Wc3b = np.ascontiguousarray(np.concatenate([-Wc, np.zeros((1, F), np.float32)], 0))
    VaT = np.ascontiguousarray(Ws[:, :F].T)
    VbT = np.ascontiguousarray(Ws[:, F:2 * F].T)
    Vc = Ws[:, 2 * F:2 * F + EDIM].T
    Vc3g = np.ascontiguousarray(np.concatenate([Vc, bs[None, :]], 0))
    Vc3d = np.ascontiguousarray(np.concatenate([-Vc, np.zeros((1, F), np.float32)], 0))
    return WaT, WbT, Wc3a, Wc3b, VaT, VbT, Vc3g, Vc3d


def kernel(gnn_in, centers, src, dst,
           Wf1, bf1, Ws1, bs1, g1, be1,
           Wf2, bf2, Ws2, bs2, g2, be2,
           _trace=False, _tmpdir=None):
    gnn_in = np.ascontiguousarray(np.asarray(gnn_in, np.float32))
    centers = np.ascontiguousarray(np.asarray(centers, np.float32))
    src = np.asarray(src, np.int32)
    dst = np.asarray(dst, np.int32)
    args = [np.asarray(a, np.float32) for a in
            (Wf1, bf1, Ws1, bs1, g1, be1, Wf2, bf2, Ws2, bs2, g2, be2)]
    (Wf1, bf1, Ws1, bs1, g1, be1, Wf2, bf2, Ws2, bs2, g2, be2) = args

    if not _canonical_edge_ok(src, dst):
        import sys
        print("kernel.py: edge index is not block-fully-connected; numpy fallback",
              file=sys.stderr)
        return _numpy_fallback(gnn_in, centers, src, dst,
                               [(Wf1, bf1, Ws1, bs1, g1, be1),
                                (Wf2, bf2, Ws2, bs2, g2, be2)])

    from concourse import bass_utils

    nc = _get_nc()

    w1 = _host_weights(Wf1, bf1, Ws1, bs1)
    w2 = _host_weights(Wf2, bf2, Ws2, bs2)
    wmap = {}
    for l, w in ((1, w1), (2, w2)):
        for n, a in zip(("WaT", "WbT", "Wc3a", "Wc3b", "VaT", "VbT", "Vc3g", "Vc3d"), w):
            wmap[f"{n}{l}"] = a
    wmap["g1"] = np.ascontiguousarray(g1[:, None])
    wmap["be1"] = np.ascontiguousarray(be1[:, None])
    wmap["g2"] = np.ascontiguousarray(g2[:, None])
    wmap["be2"] = np.ascontiguousarray(be2[:, None])

    in_maps = []
    for k in range(N_CORES):
        sl = slice(k * NODES_PC, (k + 1) * NODES_PC)
        m = dict(wmap)
        m["xT"] = np.ascontiguousarray(gnn_in[sl].T)
        m["c3"] = np.ascontiguousarray(
            np.concatenate([centers[sl].T, np.ones((1, NODES_PC), np.float32)], 0))
        in_maps.append(m)

    kw = {}
    if _trace:
        kw = dict(trace=True, tmpdir=_tmpdir)
    res = bass_utils.run_bass_kernel_spmd(nc, in_maps, core_ids=list(range(N_CORES)), **kw)

    out = np.empty((N, F), np.float32)
    for k in range(N_CORES):
        out[k * NODES_PC:(k + 1) * NODES_PC] = res.results[k]["yT"].T
    if _trace:
        _CACHE["last_res"] = res
    return out


# revision 7
# speedup vs baseline: 1.0581x; 1.0581x over previous
"""AgentGNN (2x CGConv + BN + residual + ReLU) on 8 TRN2 NeuronCores.

Self-contained: takes FULL inputs, shards 8 samples/core (data parallel),
runs a Bass/Tile kernel via run_bass_kernel_spmd, gathers FULL output.

Math: edges are fully-connected per 64-node sample and e_ij = c_i - c_j,
so  z_ij @ W.T + b  separates into per-node terms:
    alpha_i = x_i@Wa.T + c_i@Wc.T + b     (target part,  Wa = W[:, :F])
    beta_j  = x_j@Wb.T - c_j@Wc.T         (source part,  Wb = W[:, F:2F])
    msg_ij  = sigmoid(alpha_i + beta_j) * softplus(gamma_i + delta_j)
    agg_i   = sum_j msg_ij - msg_ii       (dense 64x64 incl. diag, minus diag)
BN (over all 4096 nodes, per feature) needs one cross-core AllReduce of
[sum, sumsq] per layer.

Engine mapping per layer (per core: 8 samples, pairwise = 8x[128,4096]):
  PE:  8 small projection matmuls (centers+bias folded in as K=3 term)
  DVE: outer-adds (broadcast-AP tensor_tensor), S*T multiply, j-reduce
  ACT: sigmoid; softplus as ln(1+exp(x)) (no softplus in this runtime's
       tables); table-set thrash avoided by dep-chaining ACT order
  GPSIMD: nothing (it shares its SBUF port with DVE - offload is zero-sum)
"""

import numpy as np

N_SAMPLES = 64
N_AGENTS = 64
N = N_SAMPLES * N_AGENTS          # 4096
F = 128
EDIM = 2
BN_EPS = 1e-5
N_CORES = 8
S_PC = N_SAMPLES // N_CORES       # 8 samples per core
NODES_PC = S_PC * N_AGENTS        # 512 nodes per core
GROUP = 4                         # samples per ACT table-set group

_CACHE = {}


def _build_nc():
    from concourse import bacc, mybir
    from concourse.tile import TileContext
    from concourse.tile_rust import add_dep_helper
    from concourse.bass_types import AP

    f32 = mybir.dt.float32
    AF = mybir.ActivationFunctionType
    OP = mybir.AluOpType
    AX = mybir.AxisListType

    nc = bacc.Bacc(trn_type="TRN2", target_bir_lowering=False, debug=False,
                   num_devices=N_CORES)

    xT = nc.declare_dram_parameter("xT", [F, NODES_PC], f32, isOutput=False)
    c3 = nc.declare_dram_parameter("c3", [EDIM + 1, NODES_PC], f32, isOutput=False)
    wnames = ["WaT", "WbT", "VaT", "VbT"]            # [128,128] x-side lhsT
    cnames = ["Wc3a", "Wc3b", "Vc3g", "Vc3d"]        # [3,128] centers+bias lhsT
    params = {}
    for l in (1, 2):
        for n in wnames:
            params[f"{n}{l}"] = nc.declare_dram_parameter(f"{n}{l}", [F, F], f32, isOutput=False)
        for n in cnames:
            params[f"{n}{l}"] = nc.declare_dram_parameter(f"{n}{l}", [EDIM + 1, F], f32, isOutput=False)
        params[f"g{l}"] = nc.declare_dram_parameter(f"g{l}", [F, 1], f32, isOutput=False)
        params[f"be{l}"] = nc.declare_dram_parameter(f"be{l}", [F, 1], f32, isOutput=False)
    yT = nc.declare_dram_parameter("yT", [F, NODES_PC], f32, isOutput=True)

    cc_in = {l: nc.dram_tensor(f"cc_in{l}", [F, 2], f32) for l in (1, 2)}
    cc_out = {l: nc.dram_tensor(f"cc_out{l}", [N_CORES * F, 2], f32, addr_space="Shared")
              for l in (1, 2)}

    with TileContext(nc) as tc:
        from contextlib import ExitStack
        with ExitStack() as ctx:
            io = ctx.enter_context(tc.tile_pool(name="io", bufs=1))
            wp = ctx.enter_context(tc.tile_pool(name="wp", bufs=1))
            node = ctx.enter_context(tc.tile_pool(name="node", bufs=1))
            pair = ctx.enter_context(tc.tile_pool(name="pair", bufs=1))
            psum = ctx.enter_context(tc.tile_pool(name="psum", bufs=1, space="PSUM"))
            small = ctx.enter_context(tc.tile_pool(name="small", bufs=1))

            # ---- load inputs & weights ----
            xt = io.tile([F, NODES_PC], f32, tag="xt")
            nc.sync.dma_start(xt[:], xT.ap()[:, :])
            c3t = io.tile([EDIM + 1, NODES_PC], f32, tag="c3t")
            nc.sync.dma_start(c3t[:], c3.ap()[:, :])
            wt = {}
            for l in (1, 2):
                for n in wnames:
                    t = wp.tile([F, F], f32, tag=f"{n}{l}")
                    nc.sync.dma_start(t[:], params[f"{n}{l}"].ap()[:, :])
                    wt[f"{n}{l}"] = t
                for n in cnames:
                    t = wp.tile([EDIM + 1, F], f32, tag=f"{n}{l}")
                    nc.sync.dma_start(t[:], params[f"{n}{l}"].ap()[:, :])
                    wt[f"{n}{l}"] = t
                for n in ("g", "be"):
                    t = wp.tile([F, 1], f32, tag=f"{n}{l}")
                    nc.sync.dma_start(t[:], params[f"{n}{l}"].ap()[:, :])
                    wt[f"{n}{l}"] = t

            def layer(l, x_in, x_out):
                # ---- node projections: alpha/beta/gamma/delta [128, 512] ----
                projs = []
                for pi, (wx, wc) in enumerate(zip(wnames, cnames)):
                    ps = psum.tile([F, NODES_PC], f32, tag=f"ps{pi}")
                    nc.tensor.matmul(ps[:], wt[f"{wx}{l}"][:], x_in[:],
                                     start=True, stop=False)
                    nc.tensor.matmul(ps[:], wt[f"{wc}{l}"][:], c3t[:],
                                     start=False, stop=True)
                    sb = node.tile([F, NODES_PC], f32, tag=f"proj{pi}")
                    nc.vector.tensor_copy(sb[:], ps[:])
                    projs.append(sb)
                al, be_, ga, de = projs

                agg = node.tile([F, NODES_PC], f32, tag="agg")

                act_chain = []   # enforced ACT execution order

                def act(*args, **kw):
                    i = nc.scalar.activation(*args, **kw)
                    if act_chain:
                        add_dep_helper(i.ins, act_chain[-1].ins,
                                       reason="act set grouping")
                    act_chain.append(i)
                    return i

                # ---- phase A: all P1 outer-adds, then all sigmoids (one
                # sigmoid-set table load per layer) ----
                p1s, p2s = {}, {}
                for c in range(S_PC):
                    sl = slice(c * N_AGENTS, (c + 1) * N_AGENTS)
                    a_bc = al[:, sl].broadcast_to([F, N_AGENTS, N_AGENTS])
                    b_bc = be_[:, sl].rearrange("p (o j) -> p o j", o=1) \
                        .broadcast_to([F, N_AGENTS, N_AGENTS])
                    p1 = pair.tile([F, N_AGENTS, N_AGENTS], f32, tag=f"p1_{c}")
                    nc.vector.tensor_tensor(p1[:], a_bc, b_bc, op=OP.add)
                    p1s[c] = p1
                for c in range(S_PC):
                    act(p1s[c][:], p1s[c][:], AF.Sigmoid)

                # ---- phase B: rolling per-sample P2 -> exp -> ln(1+u) ->
                # mult -> reduce -> diag-subtract (exp/ln share one set) ----
                for c in range(S_PC):
                    sl = slice(c * N_AGENTS, (c + 1) * N_AGENTS)
                    g_bc = ga[:, sl].broadcast_to([F, N_AGENTS, N_AGENTS])
                    d_bc = de[:, sl].rearrange("p (o j) -> p o j", o=1) \
                        .broadcast_to([F, N_AGENTS, N_AGENTS])
                    p2 = pair.tile([F, N_AGENTS, N_AGENTS], f32, tag=f"p2_{c % 2}")
                    nc.vector.tensor_tensor(p2[:], g_bc, d_bc, op=OP.add)
                    act(p2[:], p2[:], AF.Exp)
                    act(p2[:], p2[:], AF.Ln, bias=1.0)
                    # M = S*T in place over p1
                    nc.vector.tensor_tensor(p1s[c][:], p1s[c][:], p2[:],
                                            op=OP.mult)
                    nc.vector.tensor_reduce(agg[:, sl], p1s[c][:],
                                            axis=AX.X, op=OP.add)
                    # subtract the self-edge messages: diagonal of M
                    m = p1s[c]
                    diag = AP(tensor=m[:].tensor, offset=m[:].offset,
                              ap=[[4096, F], [65, N_AGENTS]])
                    nc.vector.tensor_tensor(agg[:, sl], agg[:, sl], diag,
                                            op=OP.subtract)

                # ---- BN stats: per-feature sum & sumsq over this core ----
                ssum = small.tile([F, 1], f32, tag="ssum")
                nc.vector.tensor_reduce(ssum[:], agg[:], axis=AX.X, op=OP.add)
                trash = node.tile([F, NODES_PC], f32, tag="trash")
                ssq = small.tile([F, 1], f32, tag="ssq")
                act(trash[:], agg[:], AF.Square, accum_out=ssq[:])

                dsum = nc.sync.dma_start(cc_in[l].ap()[:, 0:1], ssum[:])
                dsq = nc.sync.dma_start(cc_in[l].ap()[:, 1:2], ssq[:])
                ar = nc.gpsimd.collective_compute(
                    "AllGather", mybir.AluOpType.bypass,
                    replica_groups=[list(range(N_CORES))],
                    ins=[cc_in[l].ap().opt()], outs=[cc_out[l].ap().opt()])
                add_dep_helper(ar.ins, dsum.ins, reason="cc reads cc_in")
                add_dep_helper(ar.ins, dsq.ins, reason="cc reads cc_in")
                # gathered stats: [8*128, 2] -> view [128, 2, 8] and reduce
                gath = small.tile([F, 2, N_CORES], f32, tag="gath")
                din = nc.sync.dma_start(
                    gath[:], cc_out[l].ap().rearrange("(r p) c -> p c r", r=N_CORES))
                add_dep_helper(din.ins, ar.ins, reason="dma reads cc_out")
                gst = small.tile([F, 2], f32, tag="gst")
                nc.vector.tensor_reduce(gst[:], gath[:], axis=AX.X, op=OP.add)

                # ---- BN apply + residual + relu ----
                mean = small.tile([F, 1], f32, tag="mean")
                nc.vector.tensor_scalar(mean[:], gst[:, 0:1], 1.0 / N, None, op0=OP.mult)
                ex2 = small.tile([F, 1], f32, tag="ex2")
                nc.vector.tensor_scalar(ex2[:], gst[:, 1:2], 1.0 / N, None, op0=OP.mult)
                var = small.tile([F, 1], f32, tag="var")
                nc.vector.tensor_tensor(var[:], mean[:], mean[:], op=OP.mult)
                nc.vector.tensor_tensor(var[:], ex2[:], var[:], op=OP.subtract)
                nc.vector.tensor_scalar(var[:], var[:], BN_EPS, None, op0=OP.add)
                lnv = small.tile([F, 1], f32, tag="lnv")
                act(lnv[:], var[:], AF.Ln, bias=0.0)
                rstd = small.tile([F, 1], f32, tag="rstd")
                act(rstd[:], lnv[:], AF.Exp, bias=0.0, scale=-0.5)
                scal = small.tile([F, 1], f32, tag="scal")
                nc.vector.tensor_tensor(scal[:], rstd[:], wt[f"g{l}"][:], op=OP.mult)
                shift = small.tile([F, 1], f32, tag="shift")
                nc.vector.tensor_tensor(shift[:], mean[:], scal[:], op=OP.mult)
                nc.vector.tensor_tensor(shift[:], wt[f"be{l}"][:], shift[:], op=OP.subtract)
                nc.vector.tensor_scalar(x_out[:], agg[:], scal[:, 0:1], shift[:, 0:1],
                                        op0=OP.mult, op1=OP.add)
                nc.vector.tensor_tensor(x_out[:], x_out[:], x_in[:], op=OP.add)
                act(x_out[:], x_out[:], AF.Relu)

            x1 = io.tile([F, NODES_PC], f32, tag="x1")
            layer(1, xt, x1)
            x2 = io.tile([F, NODES_PC], f32, tag="x2")
            layer(2, x1, x2)
            nc.sync.dma_start(yT.ap()[:, :], x2[:])

    nc.compile()
    return nc


def _get_nc():
    if "nc" not in _CACHE:
        _CACHE["nc"] = _build_nc()
    return _CACHE["nc"]


def _canonical_edge_ok(src, dst):
    idx = np.arange(N_AGENTS)
    rows = np.repeat(idx, N_AGENTS)
    cols = np.tile(idx, N_AGENTS)
    m = rows != cols
    rows, cols = rows[m], cols[m]
    offs = (np.arange(N_SAMPLES) * N_AGENTS)[:, None]
    csrc = (rows[None, :] + offs).ravel().astype(np.int64)
    cdst = (cols[None, :] + offs).ravel().astype(np.int64)
    if src.shape != csrc.shape:
        return False
    key = np.sort(src.astype(np.int64) * N + dst.astype(np.int64))
    ckey = np.sort(csrc * N + cdst)
    return bool(np.array_equal(key, ckey))


def _numpy_fallback(gnn_in, centers, src, dst, Ws_all):
    def sig(x):
        return 1.0 / (1.0 + np.exp(-x))

    def sp(x):
        return np.log1p(np.exp(-np.abs(x))) + np.maximum(x, 0.0)

    x = gnn_in.astype(np.float64)
    e = (centers[dst] - centers[src]).astype(np.float64)
    for (Wf, bf, Wsm, bs, g, be) in Ws_all:
        z = np.concatenate([x[dst], x[src], e], axis=-1)
        msg = sig(z @ Wf.T + bf) * sp(z @ Wsm.T + bs)
        agg = np.zeros_like(x)
        np.add.at(agg, dst, msg)
        mean = agg.mean(0)
        var = agg.var(0)
        agg = (agg - mean) / np.sqrt(var + BN_EPS) * g + be
        x = np.maximum(agg + x, 0.0)
    return x.astype(np.float32)


def _host_weights(Wf, bf, Ws, bs):
    """lhsT forms for the projection matmuls."""
    WaT = np.ascontiguousarray(Wf[:, :F].T)
    WbT = np.ascontiguousarray(Wf[:, F:2 * F].T)
    Wc = Wf[:, 2 * F:2 * F + EDIM].T           # [2, 128]
    Wc3a = np.ascontiguousarray(np.concatenate([Wc, bf[None, :]], 0))
    Wc3b = np.ascontiguousarray(np.concatenate([-Wc, np.zeros((1, F), np.float32)], 0))
    VaT = np.ascontiguousarray(Ws[:, :F].T)
    VbT = np.ascontiguousarray(Ws[:, F:2 * F].T)
    Vc = Ws[:, 2 * F:2 * F + EDIM].T
    Vc3g = np.ascontiguousarray(np.concatenate([Vc, bs[None, :]], 0))
    Vc3d = np.ascontiguousarray(np.concatenate([-Vc, np.zeros((1, F), np.float32)], 0))
    return WaT, WbT, Wc3a, Wc3b, VaT, VbT, Vc3g, Vc3d


def kernel(gnn_in, centers, src, dst,
           Wf1, bf1, Ws1, bs1, g1, be1,
           Wf2, bf2, Ws2, bs2, g2, be2,
           _trace=False, _tmpdir=None):
    gnn_in = np.ascontiguousarray(np.asarray(gnn_in, np.float32))
    centers = np.ascontiguousarray(np.asarray(centers, np.float32))
    src = np.asarray(src, np.int32)
    dst = np.asarray(dst, np.int32)
    args = [np.asarray(a, np.float32) for a in
            (Wf1, bf1, Ws1, bs1, g1, be1, Wf2, bf2, Ws2, bs2, g2, be2)]
    (Wf1, bf1, Ws1, bs1, g1, be1, Wf2, bf2, Ws2, bs2, g2, be2) = args

    if not _canonical_edge_ok(src, dst):
        import sys
        print("kernel.py: edge index is not block-fully-connected; numpy fallback",
              file=sys.stderr)
        return _numpy_fallback(gnn_in, centers, src, dst,
                               [(Wf1, bf1, Ws1, bs1, g1, be1),
                                (Wf2, bf2, Ws2, bs2, g2, be2)])

    from concourse import bass_utils

    nc = _get_nc()

    w1 = _host_weights(Wf1, bf1, Ws1, bs1)
    w2 = _host_weights(Wf2, bf2, Ws2, bs2)
    wmap = {}
    for l, w in ((1, w1), (2, w2)):
        for n, a in zip(("WaT", "WbT", "Wc3a", "Wc3b", "VaT", "VbT", "Vc3g", "Vc3d"), w):
            wmap[f"{n}{l}"] = a
    wmap["g1"] = np.ascontiguousarray(g1[:, None])
    wmap["be1"] = np.ascontiguousarray(be1[:, None])
    wmap["g2"] = np.ascontiguousarray(g2[:, None])
    wmap["be2"] = np.ascontiguousarray(be2[:, None])

    in_maps = []
    for k in range(N_CORES):
        sl = slice(k * NODES_PC, (k + 1) * NODES_PC)
        m = dict(wmap)
        m["xT"] = np.ascontiguousarray(gnn_in[sl].T)
        m["c3"] = np.ascontiguousarray(
            np.concatenate([centers[sl].T, np.ones((1, NODES_PC), np.float32)], 0))
        in_maps.append(m)

    kw = {}
    if _trace:
        kw = dict(trace=True, tmpdir=_tmpdir)
    res = bass_utils.run_bass_kernel_spmd(nc, in_maps, core_ids=list(range(N_CORES)), **kw)

    out = np.empty((N, F), np.float32)
    for k in range(N_CORES):
        out[k * NODES_PC:(k + 1) * NODES_PC] = res.results[k]["yT"].T
    if _trace:
        _CACHE["last_res"] = res
    return out


# revision 9
# speedup vs baseline: 1.1259x; 1.0641x over previous
"""AgentGNN (2x CGConv + BN + residual + ReLU) on 8 TRN2 NeuronCores.

Self-contained: takes FULL inputs, shards 8 samples/core (data parallel),
runs a Bass/Tile kernel via run_bass_kernel_spmd, gathers FULL output.

Math: edges are fully-connected per 64-node sample and e_ij = c_i - c_j,
so  z_ij @ W.T + b  separates into per-node terms:
    alpha_i = x_i@Wa.T + c_i@Wc.T + b     (target part,  Wa = W[:, :F])
    beta_j  = x_j@Wb.T - c_j@Wc.T         (source part,  Wb = W[:, F:2F])
    msg_ij  = sigmoid(alpha_i + beta_j) * softplus(gamma_i + delta_j)
    agg_i   = sum_j msg_ij - msg_ii       (dense 64x64 incl. diag, minus diag)
BN (over all 4096 nodes, per feature) needs one cross-core AllReduce of
[sum, sumsq] per layer.

Engine mapping per layer (per core: 8 samples, pairwise = 8x[128,4096]):
  PE:  8 small projection matmuls (centers+bias folded in as K=3 term)
  DVE: outer-adds (broadcast-AP tensor_tensor), S*T multiply, j-reduce
  ACT: sigmoid; softplus as ln(1+exp(x)) (no softplus in this runtime's
       tables); table-set thrash avoided by dep-chaining ACT order
  GPSIMD: nothing (it shares its SBUF port with DVE - offload is zero-sum)
"""

import numpy as np

N_SAMPLES = 64
N_AGENTS = 64
N = N_SAMPLES * N_AGENTS          # 4096
F = 128
EDIM = 2
BN_EPS = 1e-5
N_CORES = 8
S_PC = N_SAMPLES // N_CORES       # 8 samples per core
NODES_PC = S_PC * N_AGENTS        # 512 nodes per core
GROUP = 4                         # samples per ACT table-set group

_CACHE = {}


def _patch_act_tables():
    """Make exp/ln resolve only in natural_log_exp_and_others so the
    table-load inserter doesn't thrash between exp_and_others and
    natural_log on every exp<->ln pair. Set-list positions (= set ids)
    are preserved; only the two duplicated functions are hidden."""
    from concourse import bacc, mybir, hw_specs

    if getattr(bacc, "_act_tables_patched", False):
        return
    AF = mybir.ActivationFunctionType
    orig = hw_specs.get_activation_tables

    def patched(arch):
        t = orig(arch)
        out = {}
        for name, s in t.items():
            s = set(s)
            if name == "exp_and_others":
                s.discard(AF.Exp)
            if name == "natural_log":
                s.discard(AF.Ln)
            out[name] = s
        return out

    bacc.get_activation_tables = patched
    bacc._act_tables_patched = True


def _build_nc():
    from concourse import bacc, mybir
    from concourse.tile import TileContext
    from concourse.tile_rust import add_dep_helper
    from concourse.bass_types import AP

    _patch_act_tables()

    f32 = mybir.dt.float32
    AF = mybir.ActivationFunctionType
    OP = mybir.AluOpType
    AX = mybir.AxisListType

    nc = bacc.Bacc(trn_type="TRN2", target_bir_lowering=False, debug=False,
                   num_devices=N_CORES)

    xT = nc.declare_dram_parameter("xT", [F, NODES_PC], f32, isOutput=False)
    c3 = nc.declare_dram_parameter("c3", [EDIM + 1, NODES_PC], f32, isOutput=False)
    wnames = ["WaT", "WbT", "VaT", "VbT"]            # [128,128] x-side lhsT
    cnames = ["Wc3a", "Wc3b", "Vc3g", "Vc3d"]        # [3,128] centers+bias lhsT
    params = {}
    for l in (1, 2):
        for n in wnames:
            params[f"{n}{l}"] = nc.declare_dram_parameter(f"{n}{l}", [F, F], f32, isOutput=False)
        for n in cnames:
            params[f"{n}{l}"] = nc.declare_dram_parameter(f"{n}{l}", [EDIM + 1, F], f32, isOutput=False)
        params[f"g{l}"] = nc.declare_dram_parameter(f"g{l}", [F, 1], f32, isOutput=False)
        params[f"be{l}"] = nc.declare_dram_parameter(f"be{l}", [F, 1], f32, isOutput=False)
    yT = nc.declare_dram_parameter("yT", [F, NODES_PC], f32, isOutput=True)

    cc_in = {l: nc.dram_tensor(f"cc_in{l}", [F, 2], f32) for l in (1, 2)}
    cc_out = {l: nc.dram_tensor(f"cc_out{l}", [N_CORES * F, 2], f32, addr_space="Shared")
              for l in (1, 2)}

    with TileContext(nc) as tc:
        from contextlib import ExitStack
        with ExitStack() as ctx:
            io = ctx.enter_context(tc.tile_pool(name="io", bufs=1))
            wp = ctx.enter_context(tc.tile_pool(name="wp", bufs=1))
            node = ctx.enter_context(tc.tile_pool(name="node", bufs=1))
            pair = ctx.enter_context(tc.tile_pool(name="pair", bufs=1))
            psum = ctx.enter_context(tc.tile_pool(name="psum", bufs=1, space="PSUM"))
            small = ctx.enter_context(tc.tile_pool(name="small", bufs=1))

            # ---- load inputs & weights ----
            xt = io.tile([F, NODES_PC], f32, tag="xt")
            nc.sync.dma_start(xt[:], xT.ap()[:, :])
            c3t = io.tile([EDIM + 1, NODES_PC], f32, tag="c3t")
            nc.sync.dma_start(c3t[:], c3.ap()[:, :])
            wt = {}
            for l in (1, 2):
                for n in wnames:
                    t = wp.tile([F, F], f32, tag=f"{n}{l}")
                    nc.sync.dma_start(t[:], params[f"{n}{l}"].ap()[:, :])
                    wt[f"{n}{l}"] = t
                for n in cnames:
                    t = wp.tile([EDIM + 1, F], f32, tag=f"{n}{l}")
                    nc.sync.dma_start(t[:], params[f"{n}{l}"].ap()[:, :])
                    wt[f"{n}{l}"] = t
                for n in ("g", "be"):
                    t = wp.tile([F, 1], f32, tag=f"{n}{l}")
                    nc.sync.dma_start(t[:], params[f"{n}{l}"].ap()[:, :])
                    wt[f"{n}{l}"] = t

            def prep_proj(l):
                # centers+bias part of the projections only needs c3t -- issue
                # early so the layer boundary only pays the x-part matmuls
                pss = []
                for pi, wc in enumerate(cnames):
                    ps = psum.tile([F, NODES_PC], f32, tag=f"ps{pi}_{l}")
                    nc.tensor.matmul(ps[:], wt[f"{wc}{l}"][:], c3t[:],
                                     start=True, stop=False)
                    pss.append(ps)
                return pss

            def layer(l, x_in, x_out, pss):
                # ---- node projections: alpha/beta/gamma/delta [128, 512] ----
                projs = []
                for pi, wx in enumerate(wnames):
                    ps = pss[pi]
                    nc.tensor.matmul(ps[:], wt[f"{wx}{l}"][:], x_in[:],
                                     start=False, stop=True)
                    sb = node.tile([F, NODES_PC], f32, tag=f"proj{pi}")
                    nc.scalar.copy(sb[:], ps[:])
                    projs.append(sb)
                al, be_, ga, de = projs

                agg = node.tile([F, NODES_PC], f32, tag="agg")

                act_chain = []   # enforced ACT execution order

                def act(*args, **kw):
                    i = nc.scalar.activation(*args, **kw)
                    if act_chain:
                        add_dep_helper(i.ins, act_chain[-1].ins,
                                       reason="act set grouping")
                    act_chain.append(i)
                    return i

                # ---- phase A: all P1 outer-adds, then all sigmoids (one
                # sigmoid-set table load per layer) ----
                p1s, p2s = {}, {}
                for c in range(S_PC):
                    sl = slice(c * N_AGENTS, (c + 1) * N_AGENTS)
                    a_bc = al[:, sl].broadcast_to([F, N_AGENTS, N_AGENTS])
                    b_bc = be_[:, sl].rearrange("p (o j) -> p o j", o=1) \
                        .broadcast_to([F, N_AGENTS, N_AGENTS])
                    p1 = pair.tile([F, N_AGENTS, N_AGENTS], f32, tag=f"p1_{c}")
                    nc.vector.tensor_tensor(p1[:], a_bc, b_bc, op=OP.add)
                    p1s[c] = p1
                for c in range(S_PC):
                    act(p1s[c][:], p1s[c][:], AF.Sigmoid)

                # ---- phase B: rolling per-sample P2 -> exp -> ln(1+u) ->
                # mult -> reduce -> diag-subtract (exp/ln share one set) ----
                for c in range(S_PC):
                    sl = slice(c * N_AGENTS, (c + 1) * N_AGENTS)
                    g_bc = ga[:, sl].broadcast_to([F, N_AGENTS, N_AGENTS])
                    d_bc = de[:, sl].rearrange("p (o j) -> p o j", o=1) \
                        .broadcast_to([F, N_AGENTS, N_AGENTS])
                    p2 = pair.tile([F, N_AGENTS, N_AGENTS], f32, tag=f"p2_{c % 2}")
                    nc.vector.tensor_tensor(p2[:], g_bc, d_bc, op=OP.add)
                    act(p2[:], p2[:], AF.Exp)
                    act(p2[:], p2[:], AF.Ln, bias=1.0)
                    # M = S*T in place over p1
                    nc.vector.tensor_tensor(p1s[c][:], p1s[c][:], p2[:],
                                            op=OP.mult)
                    nc.vector.tensor_reduce(agg[:, sl], p1s[c][:],
                                            axis=AX.X, op=OP.add)
                    # subtract the self-edge messages: diagonal of M
                    m = p1s[c]
                    diag = AP(tensor=m[:].tensor, offset=m[:].offset,
                              ap=[[4096, F], [65, N_AGENTS]])
                    nc.vector.tensor_tensor(agg[:, sl], agg[:, sl], diag,
                                            op=OP.subtract)

                # ---- BN stats: per-feature sum & sumsq over this core ----
                ssum = small.tile([F, 1], f32, tag="ssum")
                nc.vector.tensor_reduce(ssum[:], agg[:], axis=AX.X, op=OP.add)
                trash = node.tile([F, NODES_PC], f32, tag="trash")
                ssq = small.tile([F, 1], f32, tag="ssq")
                act(trash[:], agg[:], AF.Square, accum_out=ssq[:])

                dsum = nc.sync.dma_start(cc_in[l].ap()[:, 0:1], ssum[:])
                dsq = nc.sync.dma_start(cc_in[l].ap()[:, 1:2], ssq[:])
                ar = nc.gpsimd.collective_compute(
                    "AllGather", mybir.AluOpType.bypass,
                    replica_groups=[list(range(N_CORES))],
                    ins=[cc_in[l].ap().opt()], outs=[cc_out[l].ap().opt()])
                add_dep_helper(ar.ins, dsum.ins, reason="cc reads cc_in")
                add_dep_helper(ar.ins, dsq.ins, reason="cc reads cc_in")
                # gathered stats: [8*128, 2] -> view [128, 2, 8] and reduce
                gath = small.tile([F, 2, N_CORES], f32, tag="gath")
                din = nc.sync.dma_start(
                    gath[:], cc_out[l].ap().rearrange("(r p) c -> p c r", r=N_CORES))
                add_dep_helper(din.ins, ar.ins, reason="dma reads cc_out")
                gst = small.tile([F, 2], f32, tag="gst")
                nc.vector.tensor_reduce(gst[:], gath[:], axis=AX.X, op=OP.add)

                # ---- BN apply + residual + relu ----
                me2 = small.tile([F, 2], f32, tag="me2")
                nc.vector.tensor_scalar(me2[:], gst[:], 1.0 / N, None, op0=OP.mult)
                mean, ex2 = me2[:, 0:1], me2[:, 1:2]
                var = small.tile([F, 1], f32, tag="var")
                nc.vector.tensor_tensor(var[:], mean, mean, op=OP.mult)
                nc.vector.tensor_tensor(var[:], ex2, var[:], op=OP.subtract)
                lnv = small.tile([F, 1], f32, tag="lnv")
                act(lnv[:], var[:], AF.Ln, bias=eps_t[:])
                rstd = small.tile([F, 1], f32, tag="rstd")
                act(rstd[:], lnv[:], AF.Exp, bias=0.0, scale=-0.5)
                scal = small.tile([F, 1], f32, tag="scal")
                nc.vector.tensor_tensor(scal[:], rstd[:], wt[f"g{l}"][:], op=OP.mult)
                shneg = small.tile([F, 1], f32, tag="shneg")
                nc.vector.tensor_scalar(shneg[:], mean, scal[:, 0:1], wt[f"be{l}"][:][:, 0:1],
                                        op0=OP.mult, op1=OP.subtract)
                nc.vector.tensor_scalar(x_out[:], agg[:], scal[:, 0:1], shneg[:, 0:1],
                                        op0=OP.mult, op1=OP.subtract)
                nc.vector.tensor_tensor(x_out[:], x_out[:], x_in[:], op=OP.add)
                act(x_out[:], x_out[:], AF.Relu)

            eps_t = small.tile([F, 1], f32, tag="eps")
            nc.gpsimd.memset(eps_t[:], BN_EPS)
            pss1 = prep_proj(1)
            pss2 = prep_proj(2)
            x1 = io.tile([F, NODES_PC], f32, tag="x1")
            layer(1, xt, x1, pss1)
            x2 = io.tile([F, NODES_PC], f32, tag="x2")
            layer(2, x1, x2, pss2)
            nc.sync.dma_start(yT.ap()[:, :], x2[:])

    nc.compile()
    return nc


def _get_nc():
    if "nc" not in _CACHE:
        _CACHE["nc"] = _build_nc()
    return _CACHE["nc"]


def _canonical_edge_ok(src, dst):
    idx = np.arange(N_AGENTS)
    rows = np.repeat(idx, N_AGENTS)
    cols = np.tile(idx, N_AGENTS)
    m = rows != cols
    rows, cols = rows[m], cols[m]
    offs = (np.arange(N_SAMPLES) * N_AGENTS)[:, None]
    csrc = (rows[None, :] + offs).ravel().astype(np.int64)
    cdst = (cols[None, :] + offs).ravel().astype(np.int64)
    if src.shape != csrc.shape:
        return False
    key = np.sort(src.astype(np.int64) * N + dst.astype(np.int64))
    ckey = np.sort(csrc * N + cdst)
    return bool(np.array_equal(key, ckey))


def _numpy_fallback(gnn_in, centers, src, dst, Ws_all):
    def sig(x):
        return 1.0 / (1.0 + np.exp(-x))

    def sp(x):
        return np.log1p(np.exp(-np.abs(x))) + np.maximum(x, 0.0)

    x = gnn_in.astype(np.float64)
    e = (centers[dst] - centers[src]).astype(np.float64)
    for (Wf, bf, Wsm, bs, g, be) in Ws_all:
        z = np.concatenate([x[dst], x[src], e], axis=-1)
        msg = sig(z @ Wf.T + bf) * sp(z @ Wsm.T + bs)
        agg = np.zeros_like(x)
        np.add.at(agg, dst, msg)
        mean = agg.mean(0)
        var = agg.var(0)
        agg = (agg - mean) / np.sqrt(var + BN_EPS) * g + be
        x = np.maximum(agg + x, 0.0)
    return x.astype(np.float32)


def _host_weights(Wf, bf, Ws, bs):
    """lhsT forms for the projection matmuls."""
    WaT = np.ascontiguousarray(Wf[:, :F].T)
    WbT = np.ascontiguousarray(Wf[:, F:2 * F].T)
    Wc = Wf[:, 2 * F:2 * F + EDIM].T           # [2, 128]
    Wc3a = np.ascontiguousarray(np.concatenate([Wc, bf[None, :]], 0))
    Wc3b = np.ascontiguousarray(np.concatenate([-Wc, np.zeros((1, F), np.float32)], 0))
    VaT = np.ascontiguousarray(Ws[:, :F].T)
    VbT = np.ascontiguousarray(Ws[:, F:2 * F].T)
    Vc = Ws[:, 2 * F:2 * F + EDIM].T
    Vc3g = np.ascontiguousarray(np.concatenate([Vc, bs[None, :]], 0))
    Vc3d = np.ascontiguousarray(np.concatenate([-Vc, np.zeros((1, F), np.float32)], 0))
    return WaT, WbT, Wc3a, Wc3b, VaT, VbT, Vc3g, Vc3d


def kernel(gnn_in, centers, src, dst,
           Wf1, bf1, Ws1, bs1, g1, be1,
           Wf2, bf2, Ws2, bs2, g2, be2,
           _trace=False, _tmpdir=None):
    gnn_in = np.ascontiguousarray(np.asarray(gnn_in, np.float32))
    centers = np.ascontiguousarray(np.asarray(centers, np.float32))
    src = np.asarray(src, np.int32)
    dst = np.asarray(dst, np.int32)
    args = [np.asarray(a, np.float32) for a in
            (Wf1, bf1, Ws1, bs1, g1, be1, Wf2, bf2, Ws2, bs2, g2, be2)]
    (Wf1, bf1, Ws1, bs1, g1, be1, Wf2, bf2, Ws2, bs2, g2, be2) = args

    if not _canonical_edge_ok(src, dst):
        import sys
        print("kernel.py: edge index is not block-fully-connected; numpy fallback",
              file=sys.stderr)
        return _numpy_fallback(gnn_in, centers, src, dst,
                               [(Wf1, bf1, Ws1, bs1, g1, be1),
                                (Wf2, bf2, Ws2, bs2, g2, be2)])

    from concourse import bass_utils

    nc = _get_nc()

    w1 = _host_weights(Wf1, bf1, Ws1, bs1)
    w2 = _host_weights(Wf2, bf2, Ws2, bs2)
    wmap = {}
    for l, w in ((1, w1), (2, w2)):
        for n, a in zip(("WaT", "WbT", "Wc3a", "Wc3b", "VaT", "VbT", "Vc3g", "Vc3d"), w):
            wmap[f"{n}{l}"] = a
    wmap["g1"] = np.ascontiguousarray(g1[:, None])
    wmap["be1"] = np.ascontiguousarray(be1[:, None])
    wmap["g2"] = np.ascontiguousarray(g2[:, None])
    wmap["be2"] = np.ascontiguousarray(be2[:, None])

    in_maps = []
    for k in range(N_CORES):
        sl = slice(k * NODES_PC, (k + 1) * NODES_PC)
        m = dict(wmap)
        m["xT"] = np.ascontiguousarray(gnn_in[sl].T)
        m["c3"] = np.ascontiguousarray(
            np.concatenate([centers[sl].T, np.ones((1, NODES_PC), np.float32)], 0))
        in_maps.append(m)

    kw = {}
    if _trace:
        kw = dict(trace=True, tmpdir=_tmpdir)
    res = bass_utils.run_bass_kernel_spmd(nc, in_maps, core_ids=list(range(N_CORES)), **kw)

    out = np.empty((N, F), np.float32)
    for k in range(N_CORES):
        out[k * NODES_PC:(k + 1) * NODES_PC] = res.results[k]["yT"].T
    if _trace:
        _CACHE["last_res"] = res
    return out


# revision 12
# speedup vs baseline: 1.3856x; 1.2306x over previous
"""AgentGNN (2x CGConv + BN + residual + ReLU) on 8 TRN2 NeuronCores.

Self-contained: takes FULL inputs, shards 8 samples/core (data parallel),
runs a Bass/Tile kernel via run_bass_kernel_spmd, gathers FULL output.

Math: edges are fully-connected per 64-node sample and e_ij = c_i - c_j,
so  z_ij @ W.T + b  separates into per-node terms:
    alpha_i = x_i@Wa.T + c_i@Wc.T + b     (target part,  Wa = W[:, :F])
    beta_j  = x_j@Wb.T - c_j@Wc.T         (source part,  Wb = W[:, F:2F])
    msg_ij  = sigmoid(alpha_i + beta_j) * softplus(gamma_i + delta_j)
    agg_i   = sum_j msg_ij - msg_ii       (dense 64x64 incl. diag, minus diag)
BN (over all 4096 nodes, per feature) needs one cross-core AllReduce of
[sum, sumsq] per layer.

Engine mapping per layer (per core: 8 samples, pairwise = 8x[128,4096]):
  PE:  8 small projection matmuls (centers+bias folded in as K=3 term)
  DVE: outer-adds (broadcast-AP tensor_tensor), S*T multiply, j-reduce
  ACT: sigmoid; softplus as ln(1+exp(x)) (no softplus in this runtime's
       tables); table-set thrash avoided by dep-chaining ACT order
  GPSIMD: nothing (it shares its SBUF port with DVE - offload is zero-sum)
"""

import numpy as np

N_SAMPLES = 64
N_AGENTS = 64
N = N_SAMPLES * N_AGENTS          # 4096
F = 128
EDIM = 2
BN_EPS = 1e-5
N_CORES = 8
S_PC = N_SAMPLES // N_CORES       # 8 samples per core
NODES_PC = S_PC * N_AGENTS        # 512 nodes per core
GROUP = 4                         # samples per ACT table-set group

_CACHE = {}


def _register_custom_ops():
    """Two custom DVE ops:
    AGNN_MULT_CSCAN: out = running prefix-sum of (in0*in1 - s0) -- fuses the
      S*T multiply with the j-reduction (segment sums recovered by
      differencing prefix values at segment ends; s0 = per-feature mean
      message keeps the prefix small so fp32 rounding stays tiny).
    AGNN_DIFF_ADD: out = in0 - in1 + s0 (the segment diff, re-adding 64*mean).
    """
    import numpy as _np
    from concourse import dve_ops as D

    if getattr(D, "_agnn_ops", None):
        return D._agnn_ops
    from concourse.dve_spec import Spec, Src0, Src1, C0, AluOp, scan, lower
    from concourse.dve_uop import DveOpSpec

    def ref_mult_scan(in0, in1, s0, s1, imm2):
        prod = (in0.astype(_np.float32) * in1 - s0).astype(_np.float32)
        return _np.cumsum(prod.reshape(prod.shape[0], -1), 1).astype(
            _np.float32).reshape(in0.shape)

    def ref_diff_add(in0, in1, s0, s1, imm2):
        return (in0.astype(_np.float32) - in1 + s0).astype(_np.float32)

    def make(name, spec, subdim):
        row = D._CUSTOM_DVE_ROW_BASE + len(D.OPS)
        D._SUB_OPCODE_FOR_NAME[name] = row
        shas = {}
        for ver in ("v3", "v4"):
            u = lower(spec, ver=ver)
            shas[ver] = DveOpSpec(name=name, opcode=row, uops=u, rd1_en=True).sha(ver)
        op = D.DveOp(name, spec, subdim=subdim, uops_sha=shas)
        D.OPS.append(op)
        D.CUSTOM_DVE_SPECS[name] = spec
        return op

    sc = Spec(body=scan(AluOp.ADD, Src0 * Src1 - C0), reference=ref_mult_scan)
    df = Spec(body=Src0 - Src1 + C0, reference=ref_diff_add)
    D._agnn_ops = (make("AGNN_MULT_CSCAN", sc, True),
                   make("AGNN_DIFF_ADD", df, False))
    return D._agnn_ops


def _patch_act_tables():
    """Make exp/ln resolve only in natural_log_exp_and_others so the
    table-load inserter doesn't thrash between exp_and_others and
    natural_log on every exp<->ln pair. Set-list positions (= set ids)
    are preserved; only the two duplicated functions are hidden."""
    from concourse import bacc, mybir, hw_specs

    if getattr(bacc, "_act_tables_patched", False):
        return
    AF = mybir.ActivationFunctionType
    orig = hw_specs.get_activation_tables

    def patched(arch):
        t = orig(arch)
        out = {}
        for name, s in t.items():
            s = set(s)
            if name == "exp_and_others":
                s.discard(AF.Exp)
            if name == "natural_log":
                s.discard(AF.Ln)
            out[name] = s
        return out

    bacc.get_activation_tables = patched
    bacc._act_tables_patched = True


def _build_nc():
    from concourse import bacc, mybir
    from concourse.tile import TileContext
    from concourse.tile_rust import add_dep_helper
    from concourse.bass_types import AP

    _patch_act_tables()
    OP_SCAN, OP_DIFF = _register_custom_ops()

    f32 = mybir.dt.float32
    AF = mybir.ActivationFunctionType
    OP = mybir.AluOpType
    AX = mybir.AxisListType

    nc = bacc.Bacc(trn_type="TRN2", target_bir_lowering=False, debug=False,
                   num_devices=N_CORES)

    xT = nc.declare_dram_parameter("xT", [F, NODES_PC], f32, isOutput=False)
    c3 = nc.declare_dram_parameter("c3", [EDIM + 1, NODES_PC], f32, isOutput=False)
    wnames = ["WaT", "WbT", "VaT", "VbT"]            # [128,128] x-side lhsT
    cnames = ["Wc3a", "Wc3b", "Vc3g", "Vc3d"]        # [3,128] centers+bias lhsT
    params = {}
    for l in (1, 2):
        for n in wnames:
            params[f"{n}{l}"] = nc.declare_dram_parameter(f"{n}{l}", [F, F], f32, isOutput=False)
        params[f"WcAll{l}"] = nc.declare_dram_parameter(f"WcAll{l}", [EDIM + 1, 4 * F], f32, isOutput=False)
        params[f"g{l}"] = nc.declare_dram_parameter(f"g{l}", [F, 1], f32, isOutput=False)
        params[f"be{l}"] = nc.declare_dram_parameter(f"be{l}", [F, 1], f32, isOutput=False)
    yT = nc.declare_dram_parameter("yT", [F, NODES_PC], f32, isOutput=True)

    cc_in = {l: nc.dram_tensor(f"cc_in{l}", [F, 2], f32) for l in (1, 2)}
    cc_out = {l: nc.dram_tensor(f"cc_out{l}", [N_CORES * F, 2], f32, addr_space="Shared")
              for l in (1, 2)}

    with TileContext(nc) as tc:
        from contextlib import ExitStack
        with ExitStack() as ctx:
            io = ctx.enter_context(tc.tile_pool(name="io", bufs=1))
            wp = ctx.enter_context(tc.tile_pool(name="wp", bufs=1))
            node = ctx.enter_context(tc.tile_pool(name="node", bufs=1))
            pair = ctx.enter_context(tc.tile_pool(name="pair", bufs=1))
            psum = ctx.enter_context(tc.tile_pool(name="psum", bufs=1, space="PSUM"))
            small = ctx.enter_context(tc.tile_pool(name="small", bufs=1))

            # ---- load inputs & weights ----
            xt = io.tile([F, NODES_PC], f32, tag="xt")
            nc.sync.dma_start(xt[:], xT.ap()[:, :])
            c3t = io.tile([EDIM + 1, NODES_PC], f32, tag="c3t")
            nc.sync.dma_start(c3t[:], c3.ap()[:, :])
            wt = {}
            for l in (1, 2):
                for n in wnames:
                    t = wp.tile([F, F], f32, tag=f"{n}{l}")
                    nc.sync.dma_start(t[:], params[f"{n}{l}"].ap()[:, :])
                    wt[f"{n}{l}"] = t
                t = wp.tile([EDIM + 1, 4 * F], f32, tag=f"WcAll{l}")
                nc.sync.dma_start(t[:], params[f"WcAll{l}"].ap()[:, :])
                wt[f"WcAll{l}"] = t
                for n in ("g", "be"):
                    t = wp.tile([F, 1], f32, tag=f"{n}{l}")
                    nc.sync.dma_start(t[:], params[f"{n}{l}"].ap()[:, :])
                    wt[f"{n}{l}"] = t

            def prep_proj(l):
                # centers+bias part of the projections only needs c3t -- issue
                # early so the layer boundary only pays the x-part matmuls
                pss = []
                for pi in range(4):
                    ps = psum.tile([F, NODES_PC], f32, tag=f"ps{pi}_{l}")
                    nc.tensor.matmul(ps[:], wt[f"WcAll{l}"][:][:, pi * F:(pi + 1) * F],
                                     c3t[:], start=True, stop=False)
                    pss.append(ps)
                return pss

            def layer(l, x_in, x_out, pss):
                # ---- node projections: alpha/beta/gamma/delta [128, 512] ----
                projs = []
                for pi, wx in enumerate(wnames):
                    ps = pss[pi]
                    nc.tensor.matmul(ps[:], wt[f"{wx}{l}"][:], x_in[:],
                                     start=False, stop=True)
                    sb = node.tile([F, NODES_PC], f32, tag=f"proj{pi}")
                    nc.scalar.copy(sb[:], ps[:])
                    projs.append(sb)
                al, be_, ga, de = projs

                agg = node.tile([F, NODES_PC], f32, tag="agg")

                act_chain = []   # enforced ACT execution order

                def act(*args, **kw):
                    i = nc.scalar.activation(*args, **kw)
                    if act_chain:
                        add_dep_helper(i.ins, act_chain[-1].ins,
                                       reason="act set grouping")
                    act_chain.append(i)
                    return i

                # ---- diagonal (self-edge) messages; their mean is also the
                # centering constant K for the fused scan ----
                d1 = node.tile([F, NODES_PC], f32, tag="d1")
                nc.vector.tensor_tensor(d1[:], al[:], be_[:], op=OP.add)
                d2 = node.tile([F, NODES_PC], f32, tag="d2")
                nc.vector.tensor_tensor(d2[:], ga[:], de[:], op=OP.add)

                # ---- phase A: all P1 outer-adds, then all sigmoids (one
                # sigmoid-set table load per layer) ----
                p1s = {}
                for c in range(S_PC):
                    sl = slice(c * N_AGENTS, (c + 1) * N_AGENTS)
                    a_bc = al[:, sl].broadcast_to([F, N_AGENTS, N_AGENTS])
                    b_bc = be_[:, sl].rearrange("p (o j) -> p o j", o=1) \
                        .broadcast_to([F, N_AGENTS, N_AGENTS])
                    p1 = pair.tile([F, N_AGENTS, N_AGENTS], f32, tag=f"p1_{c}")
                    nc.vector.tensor_tensor(p1[:], a_bc, b_bc, op=OP.add)
                    p1s[c] = p1
                act(d1[:], d1[:], AF.Sigmoid)
                for c in range(S_PC):
                    act(p1s[c][:], p1s[c][:], AF.Sigmoid)
                act(d2[:], d2[:], AF.Exp)
                act(d2[:], d2[:], AF.Ln, bias=1.0)
                # dM = dS*dT; K = mean(dM), K64 = 64*K
                nc.vector.tensor_tensor(d1[:], d1[:], d2[:], op=OP.mult)
                kt = small.tile([F, 1], f32, tag="kt")
                nc.vector.tensor_reduce(kt[:], d1[:], axis=AX.X, op=OP.add)
                km = small.tile([F, 2], f32, tag="km")
                nc.vector.tensor_scalar(km[:, 0:1], kt[:], 1.0 / NODES_PC, None, op0=OP.mult)
                nc.vector.tensor_scalar(km[:, 1:2], kt[:], float(N_AGENTS) / NODES_PC, None, op0=OP.mult)

                # prefix-sum scratch: one [64+1] row per sample, col 0 = 0
                pref = node.tile([F, S_PC, N_AGENTS + 1], f32, tag="pref")
                nc.gpsimd.memset(pref[:, :, 0:1], 0.0)

                # ---- phase B: rolling per-sample P2 -> exp -> ln(1+u) ->
                # fused mult+centered-scan -> segment diff ----
                for c in range(S_PC):
                    sl = slice(c * N_AGENTS, (c + 1) * N_AGENTS)
                    g_bc = ga[:, sl].broadcast_to([F, N_AGENTS, N_AGENTS])
                    d_bc = de[:, sl].rearrange("p (o j) -> p o j", o=1) \
                        .broadcast_to([F, N_AGENTS, N_AGENTS])
                    p2 = pair.tile([F, N_AGENTS, N_AGENTS], f32, tag=f"p2_{c % 2}")
                    nc.vector.tensor_tensor(p2[:], g_bc, d_bc, op=OP.add)
                    act(p2[:], p2[:], AF.Exp)
                    act(p2[:], p2[:], AF.Ln, bias=1.0)
                    # prefix over (S*T - K); write only segment-end values
                    scan_out = pref[:, c, 1:1 + N_AGENTS] \
                        .rearrange("p (i o) -> p i o", o=1) \
                        .broadcast_to([F, N_AGENTS, N_AGENTS])
                    nc.vector._custom_dve(
                        OP_SCAN, out=scan_out,
                        in0=p1s[c][:].rearrange("p i j -> p (i j)"),
                        in1=p2[:].rearrange("p i j -> p (i j)"),
                        s0=km[:, 0:1])
                    # agg[i] = pref[i] - pref[i-1] + 64*K
                    nc.vector._custom_dve(
                        OP_DIFF, out=agg[:, sl],
                        in0=pref[:, c, 1:1 + N_AGENTS],
                        in1=pref[:, c, 0:N_AGENTS],
                        s0=km[:, 1:2])
                # subtract self-edge messages
                nc.vector.tensor_tensor(agg[:], agg[:], d1[:], op=OP.subtract)

                # ---- BN stats: per-feature sum & sumsq over this core ----
                ssum = small.tile([F, 1], f32, tag="ssum")
                nc.vector.tensor_reduce(ssum[:], agg[:], axis=AX.X, op=OP.add)
                trash = node.tile([F, NODES_PC], f32, tag="proj0")
                ssq = small.tile([F, 1], f32, tag="ssq")
                act(trash[:], agg[:], AF.Square, accum_out=ssq[:])

                dsum = nc.sync.dma_start(cc_in[l].ap()[:, 0:1], ssum[:])
                dsq = nc.sync.dma_start(cc_in[l].ap()[:, 1:2], ssq[:])
                ar = nc.gpsimd.collective_compute(
                    "AllGather", mybir.AluOpType.bypass,
                    replica_groups=[list(range(N_CORES))],
                    ins=[cc_in[l].ap().opt()], outs=[cc_out[l].ap().opt()])
                add_dep_helper(ar.ins, dsum.ins, reason="cc reads cc_in")
                add_dep_helper(ar.ins, dsq.ins, reason="cc reads cc_in")
                # gathered stats: [8*128, 2] -> view [128, 2, 8] and reduce
                gath = small.tile([F, 2, N_CORES], f32, tag="gath")
                din = nc.sync.dma_start(
                    gath[:], cc_out[l].ap().rearrange("(r p) c -> p c r", r=N_CORES))
                add_dep_helper(din.ins, ar.ins, reason="dma reads cc_out")
                gst = small.tile([F, 2], f32, tag="gst")
                nc.vector.tensor_reduce(gst[:], gath[:], axis=AX.X, op=OP.add)

                # ---- BN apply + residual + relu ----
                me2 = small.tile([F, 2], f32, tag="me2")
                nc.vector.tensor_scalar(me2[:], gst[:], 1.0 / N, None, op0=OP.mult)
                mean, ex2 = me2[:, 0:1], me2[:, 1:2]
                var = small.tile([F, 1], f32, tag="var")
                nc.vector.tensor_tensor(var[:], mean, mean, op=OP.mult)
                nc.vector.tensor_tensor(var[:], ex2, var[:], op=OP.subtract)
                lnv = small.tile([F, 1], f32, tag="lnv")
                act(lnv[:], var[:], AF.Ln, bias=eps_t[:])
                rstd = small.tile([F, 1], f32, tag="rstd")
                act(rstd[:], lnv[:], AF.Exp, bias=0.0, scale=-0.5)
                scal = small.tile([F, 1], f32, tag="scal")
                nc.vector.tensor_tensor(scal[:], rstd[:], wt[f"g{l}"][:], op=OP.mult)
                shneg = small.tile([F, 1], f32, tag="shneg")
                nc.vector.tensor_scalar(shneg[:], mean, scal[:, 0:1], wt[f"be{l}"][:][:, 0:1],
                                        op0=OP.mult, op1=OP.subtract)
                nc.vector.tensor_scalar(x_out[:], agg[:], scal[:, 0:1], shneg[:, 0:1],
                                        op0=OP.mult, op1=OP.subtract)
                nc.vector.tensor_tensor(x_out[:], x_out[:], x_in[:], op=OP.add)
                act(x_out[:], x_out[:], AF.Relu)

            eps_t = small.tile([F, 1], f32, tag="eps")
            nc.gpsimd.memset(eps_t[:], BN_EPS)
            pss1 = prep_proj(1)
            pss2 = prep_proj(2)
            x1 = io.tile([F, NODES_PC], f32, tag="x1")
            layer(1, xt, x1, pss1)
            x2 = io.tile([F, NODES_PC], f32, tag="x2")
            layer(2, x1, x2, pss2)
            nc.sync.dma_start(yT.ap()[:, :], x2[:])

    nc.compile()
    return nc


def _get_nc():
    if "nc" not in _CACHE:
        _CACHE["nc"] = _build_nc()
    return _CACHE["nc"]


def _canonical_edge_ok(src, dst):
    idx = np.arange(N_AGENTS)
    rows = np.repeat(idx, N_AGENTS)
    cols = np.tile(idx, N_AGENTS)
    m = rows != cols
    rows, cols = rows[m], cols[m]
    offs = (np.arange(N_SAMPLES) * N_AGENTS)[:, None]
    csrc = (rows[None, :] + offs).ravel().astype(np.int64)
    cdst = (cols[None, :] + offs).ravel().astype(np.int64)
    if src.shape != csrc.shape:
        return False
    key = np.sort(src.astype(np.int64) * N + dst.astype(np.int64))
    ckey = np.sort(csrc * N + cdst)
    return bool(np.array_equal(key, ckey))


def _numpy_fallback(gnn_in, centers, src, dst, Ws_all):
    def sig(x):
        return 1.0 / (1.0 + np.exp(-x))

    def sp(x):
        return np.log1p(np.exp(-np.abs(x))) + np.maximum(x, 0.0)

    x = gnn_in.astype(np.float64)
    e = (centers[dst] - centers[src]).astype(np.float64)
    for (Wf, bf, Wsm, bs, g, be) in Ws_all:
        z = np.concatenate([x[dst], x[src], e], axis=-1)
        msg = sig(z @ Wf.T + bf) * sp(z @ Wsm.T + bs)
        agg = np.zeros_like(x)
        np.add.at(agg, dst, msg)
        mean = agg.mean(0)
        var = agg.var(0)
        agg = (agg - mean) / np.sqrt(var + BN_EPS) * g + be
        x = np.maximum(agg + x, 0.0)
    return x.astype(np.float32)


def _host_weights(Wf, bf, Ws, bs):
    """lhsT forms for the projection matmuls."""
    WaT = np.ascontiguousarray(Wf[:, :F].T)
    WbT = np.ascontiguousarray(Wf[:, F:2 * F].T)
    Wc = Wf[:, 2 * F:2 * F + EDIM].T           # [2, 128]
    z = np.zeros((1, F), np.float32)
    Wc3a = np.concatenate([Wc, bf[None, :]], 0)
    Wc3b = np.concatenate([-Wc, z], 0)
    VaT = np.ascontiguousarray(Ws[:, :F].T)
    VbT = np.ascontiguousarray(Ws[:, F:2 * F].T)
    Vc = Ws[:, 2 * F:2 * F + EDIM].T
    Vc3g = np.concatenate([Vc, bs[None, :]], 0)
    Vc3d = np.concatenate([-Vc, z], 0)
    WcAll = np.ascontiguousarray(np.concatenate([Wc3a, Wc3b, Vc3g, Vc3d], 1))
    return WaT, WbT, VaT, VbT, WcAll


def kernel(gnn_in, centers, src, dst,
           Wf1, bf1, Ws1, bs1, g1, be1,
           Wf2, bf2, Ws2, bs2, g2, be2,
           _trace=False, _tmpdir=None):
    gnn_in = np.ascontiguousarray(np.asarray(gnn_in, np.float32))
    centers = np.ascontiguousarray(np.asarray(centers, np.float32))
    src = np.asarray(src, np.int32)
    dst = np.asarray(dst, np.int32)
    args = [np.asarray(a, np.float32) for a in
            (Wf1, bf1, Ws1, bs1, g1, be1, Wf2, bf2, Ws2, bs2, g2, be2)]
    (Wf1, bf1, Ws1, bs1, g1, be1, Wf2, bf2, Ws2, bs2, g2, be2) = args

    if not _canonical_edge_ok(src, dst):
        import sys
        print("kernel.py: edge index is not block-fully-connected; numpy fallback",
              file=sys.stderr)
        return _numpy_fallback(gnn_in, centers, src, dst,
                               [(Wf1, bf1, Ws1, bs1, g1, be1),
                                (Wf2, bf2, Ws2, bs2, g2, be2)])

    from concourse import bass_utils

    nc = _get_nc()

    w1 = _host_weights(Wf1, bf1, Ws1, bs1)
    w2 = _host_weights(Wf2, bf2, Ws2, bs2)
    wmap = {}
    for l, w in ((1, w1), (2, w2)):
        for n, a in zip(("WaT", "WbT", "VaT", "VbT", "WcAll"), w):
            wmap[f"{n}{l}"] = a
    wmap["g1"] = np.ascontiguousarray(g1[:, None])
    wmap["be1"] = np.ascontiguousarray(be1[:, None])
    wmap["g2"] = np.ascontiguousarray(g2[:, None])
    wmap["be2"] = np.ascontiguousarray(be2[:, None])

    in_maps = []
    for k in range(N_CORES):
        sl = slice(k * NODES_PC, (k + 1) * NODES_PC)
        m = dict(wmap)
        m["xT"] = np.ascontiguousarray(gnn_in[sl].T)
        m["c3"] = np.ascontiguousarray(
            np.concatenate([centers[sl].T, np.ones((1, NODES_PC), np.float32)], 0))
        in_maps.append(m)

    kw = {}
    if _trace:
        kw = dict(trace=True, tmpdir=_tmpdir)
    res = bass_utils.run_bass_kernel_spmd(nc, in_maps, core_ids=list(range(N_CORES)), **kw)

    out = np.empty((N, F), np.float32)
    for k in range(N_CORES):
        out[k * NODES_PC:(k + 1) * NODES_PC] = res.results[k]["yT"].T
    if _trace:
        _CACHE["last_res"] = res
    return out
